# revision 18
# baseline (speedup 1.0000x reference)
"""BitLinear (BitNet 1.58-bit ternary) distributed Trainium2 kernel.

Reference semantics:
    scale = max(mean(|w|), 1e-5)
    w_q   = sign(w) * (|w| > scale/3)          # ternary {-1, 0, 1}
    out   = (x @ w_q.T) * scale                # x: [4, 2048, 2048], w: [2048, 2048]

Sharding: data-parallel over tokens (1024 of 8192 per core), weight
replicated; each core computes the scale locally, so there are no
collectives (cross-core sync points absorb the harness' launch skew
and invite power throttling). Host-side prep transposes both operands
so the contraction dim (in_features) lands on SBUF partitions and
pre-casts x to bf16.

scale depends on every byte of w, so w is streamed twice; the second
stream is cut to 6 of 16 K-tiles by keeping the last 10 resident in
SBUF across the scale computation (x is streamed per-m-tile through a
small rotating buffer to make room). The cross-partition total is
summed and broadcast to all 128 partitions with a single ones-matmul.

Quantization: ternary, computed doubled so it is exact in bf16:
  ACT path:  wq2 = Sign(w + t) + Sign(w - t)            in {-2, 0, 2}
  DVE path:  wq2 = 2*(w > t) - 2*(w < -t)               in {-2, 0, 2}
with t = scale/3; alternating per K-tile to balance engines. The
missing 1/2 is folded into the output scaling (psum * scale/2).
"""

import sys

sys.path.insert(0, "/opt/trn_rl_repo")

import numpy as np

N_CORES = 8
B, S, D = 4, 2048, 2048        # x: [B, S, D]
OUT = 2048                     # out_features
TOK = B * S                    # 8192 tokens
TPC = TOK // N_CORES           # 1024 tokens per core
KT = D // 128                  # 16 K-tiles of 128
MT = TPC // 128                # 8 M-tiles per core
NT = OUT // 512                # 4 N-tiles of 512
N_ELEM = float(D * OUT)        # elements of w
EPS = 1e-5
M_P1 = 2                       # m-tiles in the k-outer first phase
RESIDENT = 7                   # w K-tiles kept in SBUF across the scale calc


def build_kernel():
    from concourse import bacc, tile, mybir

    f32 = mybir.dt.float32
    bf16 = mybir.dt.bfloat16
    Alu = mybir.AluOpType
    Act = mybir.ActivationFunctionType
    X = mybir.AxisListType.X

    nc = bacc.Bacc(None, target_bir_lowering=False)
    x_ext = nc.declare_dram_parameter("x", [TPC, D], bf16, isOutput=False)
    w_ext = nc.declare_dram_parameter("weight", [D, OUT], f32, isOutput=False)
    out_ext = nc.declare_dram_parameter("out", [TPC, OUT], f32, isOutput=True)

    with tile.TileContext(nc) as tc:
        with (
            tc.tile_pool(name="persist", bufs=1) as persist,
            tc.tile_pool(name="wf32", bufs=10) as wf32_pool,
            tc.tile_pool(name="xbuf", bufs=4) as xbuf_pool,
            tc.tile_pool(name="sgn", bufs=4) as sgn_pool,
            tc.tile_pool(name="outp", bufs=1) as out_pool,
            tc.tile_pool(name="psum", bufs=8, space="PSUM") as psum_pool,
        ):
            wq = persist.tile([128, KT, OUT], bf16)      # quantized w^T (doubled)
            ones = persist.tile([128, 128], f32)
            partials = persist.tile([128, KT], f32)
            tot = persist.tile([128, 1], f32)
            scale_sb = persist.tile([128, 1], f32)
            t_pos = persist.tile([128, 1], f32)
            t_neg = persist.tile([128, 1], f32)
            s_half = persist.tile([128, 1], f32)

            nc.vector.memset(ones[:], 1.0)
            # PE warm-up: fetch PE's IRAM block + park the sequencer early so
            # the scale-broadcast matmul fires the moment its input is ready
            warm = psum_pool.tile([128, 512], f32, tag="psum", name="warm")
            nc.tensor.matmul(
                warm[:, 0:1], ones[:], ones[:, 0:1], start=True, stop=True
            )

            def x_dma(m):
                xb = xbuf_pool.tile([128, KT, 128], bf16, tag="xbuf", name=f"xb{m}")
                nc.sync.dma_start(
                    xb[:],
                    x_ext[m * 128 : (m + 1) * 128, :].rearrange(
                        "p (k c) -> p k c", k=KT
                    ),
                )
                return xb

            def w_dma(k, nm):
                wt = wf32_pool.tile([128, OUT], f32, tag="wf32", name=nm)
                nc.sync.dma_start(wt[:], w_ext[k * 128 : (k + 1) * 128, :])
                return wt

            # ---- stream 1: all of w, |w| row-sums; keep last RESIDENT tiles ----
            res_tiles = {}
            for k in range(KT):
                wt = w_dma(k, f"wt{k}")
                nc.vector.tensor_reduce(
                    partials[:, k : k + 1], wt[:], axis=X, op=Alu.add,
                    apply_absolute_value=True,
                )
                if k >= KT - RESIDENT:
                    res_tiles[k] = wt

            # ---- scale: sum partials, broadcast via ones-matmul ----
            nc.vector.tensor_reduce(tot[:], partials[:], axis=X, op=Alu.add)
            pbc = psum_pool.tile([128, 512], f32, tag="psum", name="pbc")
            nc.tensor.matmul(pbc[:, 0:1], ones[:], tot[:], start=True, stop=True)
            nc.vector.tensor_scalar(
                scale_sb[:], pbc[:, 0:1], 1.0 / N_ELEM, EPS, Alu.mult, Alu.max
            )
            nc.vector.tensor_scalar(t_pos[:], scale_sb[:], 1.0 / 3.0, None, Alu.mult)
            nc.vector.tensor_scalar(t_neg[:], scale_sb[:], -1.0 / 3.0, None, Alu.mult)
            nc.vector.tensor_scalar(s_half[:], scale_sb[:], 0.5, None, Alu.mult)

            # ---- quantize one K-tile (doubled ternary), hybrid ACT/DVE ----
            GPS_TILES = {11, 15, 3, 7}
            ACT_TILES = {9, 10, 12, 14, 0, 2, 4}

            def quantize(k, wt):
                if k in GPS_TILES:
                    pos = sgn_pool.tile([128, OUT], bf16, tag="sgn", name=f"gp_{k}")
                    neg = sgn_pool.tile([128, OUT], bf16, tag="sgn", name=f"gn_{k}")
                    nc.gpsimd.tensor_scalar(
                        pos[:], wt[:], t_pos[:, 0:1], 2.0, Alu.is_gt, Alu.mult
                    )
                    nc.gpsimd.tensor_scalar(
                        neg[:], wt[:], t_neg[:, 0:1], 2.0, Alu.is_lt, Alu.mult
                    )
                    nc.gpsimd.tensor_tensor(wq[:, k, :], pos[:], neg[:], Alu.subtract)
                elif k in ACT_TILES:
                    s1 = sgn_pool.tile([128, OUT], bf16, tag="sgn", name=f"s1_{k}")
                    s2 = sgn_pool.tile([128, OUT], bf16, tag="sgn", name=f"s2_{k}")
                    nc.scalar.activation(s1[:], wt[:], Act.Sign, bias=t_pos[:, 0:1])
                    nc.scalar.activation(s2[:], wt[:], Act.Sign, bias=t_neg[:, 0:1])
                    nc.vector.tensor_tensor(wq[:, k, :], s1[:], s2[:], Alu.add)
                else:
                    neg = sgn_pool.tile([128, OUT], bf16, tag="sgn", name=f"n_{k}")
                    nc.vector.tensor_scalar(
                        wq[:, k, :], wt[:], t_pos[:, 0:1], 2.0, Alu.is_gt, Alu.mult
                    )
                    nc.vector.tensor_scalar(
                        neg[:], wt[:], t_neg[:, 0:1], 2.0, Alu.is_lt, Alu.mult
                    )
                    nc.vector.tensor_tensor(
                        wq[:, k, :], wq[:, k, :], neg[:], Alu.subtract
                    )

            # resident tiles first (data already on-chip when scale lands)
            for k in range(KT - RESIDENT, KT):
                quantize(k, res_tiles[k])

            # x for matmul phase 1, then the w re-stream interleaved with the
            # rest of x (phase-2 m order)
            xbufs = {m: x_dma(m) for m in range(M_P1)}
            xq = list(range(M_P1, M_P1 + 2))       # needed right at phase-2 start
            xlate = list(range(M_P1 + 2, MT))      # needed deep into phase-2
            for k in range(KT - RESIDENT):
                wt = w_dma(k, f"wr{k}")
                quantize(k, wt)
                if xq:
                    m_next = xq.pop(0)
                    xbufs[m_next] = x_dma(m_next)
            for m in xq + xlate:
                xbufs[m] = x_dma(m)

            # ---- matmul: out[m,n] = sum_k x[k,m].T @ wq[k,n] ----
            # phase-1 k order matches quant availability order
            korder = list(range(KT - RESIDENT, KT)) + list(range(KT - RESIDENT))

            def do_mtile(ms, ks):
                psums = [
                    psum_pool.tile([128, 512], f32, tag="psum", name=f"ps{i}")
                    for i in range(NT * len(ms))
                ]
                for ki, k in enumerate(ks):
                    for mi, m in enumerate(ms):
                        for n in range(NT):
                            nc.tensor.matmul(
                                psums[mi * NT + n][:],
                                xbufs[m][:, k, :],
                                wq[:, k, n * 512 : (n + 1) * 512],
                                start=(ki == 0),
                                stop=(ki == KT - 1),
                            )
                for mi, m in enumerate(ms):
                    ot = out_pool.tile([128, OUT], f32, tag="outp", name=f"ot{m}")
                    for n in range(NT):
                        nc.scalar.activation(
                            ot[:, n * 512 : (n + 1) * 512],
                            psums[mi * NT + n][:],
                            Act.Copy,
                            scale=s_half[:, 0:1],
                        )
                        nc.sync.dma_start(
                            out_ext[m * 128 : (m + 1) * 128, n * 512 : (n + 1) * 512],
                            ot[:, n * 512 : (n + 1) * 512],
                        )

            do_mtile(list(range(M_P1)), korder)
            for m in range(M_P1, MT):
                do_mtile([m], list(range(KT)))

    nc.finalize()
    return nc


_NC_CACHE = None


def kernel(x, weight):
    global _NC_CACHE
    import ml_dtypes
    from concourse.bass_utils import run_bass_kernel_spmd

    x = np.asarray(x, dtype=np.float32).reshape(TOK, D)
    weight = np.asarray(weight, dtype=np.float32)
    wT = np.ascontiguousarray(weight.T)                      # [in, out] f32
    in_maps = []
    for i in range(N_CORES):
        shard_t = x[i * TPC : (i + 1) * TPC].T                      # [in, tok]
        tiled = (
            shard_t.reshape(KT, 128, MT, 128)
            .transpose(2, 1, 0, 3)
            .reshape(MT * 128, KT * 128)
        )
        in_maps.append(
            {"x": np.ascontiguousarray(tiled).astype(ml_dtypes.bfloat16),
             "weight": wT}
        )

    if _NC_CACHE is None:
        _NC_CACHE = build_kernel()
    res = run_bass_kernel_spmd(_NC_CACHE, in_maps, core_ids=list(range(N_CORES)))
    outs = [res.results[i]["out"] for i in range(N_CORES)]
    return np.concatenate(outs, axis=0).reshape(B, S, OUT).astype(np.float32)


# revision 19
# speedup vs baseline: 2.8650x; 2.8650x over previous
"""BitLinear (BitNet 1.58-bit ternary) distributed Trainium2 kernel.

Reference semantics:
    scale = max(mean(|w|), 1e-5)
    w_q   = sign(w) * (|w| > scale/3)          # ternary {-1, 0, 1}
    out   = (x @ w_q.T) * scale                # x: [4, 2048, 2048], w: [2048, 2048]

Sharding: data-parallel over tokens (1024 of 8192 per core), weight
replicated; each core computes the scale locally, so there are no
collectives (cross-core sync points absorb the harness' launch skew
and invite power throttling). Host-side prep transposes both operands
so the contraction dim (in_features) lands on SBUF partitions and
pre-casts x to bf16.

scale depends on every byte of w, so w is streamed twice; the second
stream is cut to 6 of 16 K-tiles by keeping the last 10 resident in
SBUF across the scale computation (x is streamed per-m-tile through a
small rotating buffer to make room). The cross-partition total is
summed and broadcast to all 128 partitions with a single ones-matmul.

Quantization: ternary, computed doubled so it is exact in bf16:
  ACT path:  wq2 = Sign(w + t) + Sign(w - t)            in {-2, 0, 2}
  DVE path:  wq2 = 2*(w > t) - 2*(w < -t)               in {-2, 0, 2}
with t = scale/3; alternating per K-tile to balance engines. The
missing 1/2 is folded into the output scaling (psum * scale/2).
"""

import sys

sys.path.insert(0, "/opt/trn_rl_repo")

import numpy as np

N_CORES = 8
B, S, D = 4, 2048, 2048        # x: [B, S, D]
OUT = 2048                     # out_features
TOK = B * S                    # 8192 tokens
TPC = TOK // N_CORES           # 1024 tokens per core
KT = D // 128                  # 16 K-tiles of 128
MT = TPC // 128                # 8 M-tiles per core
NT = OUT // 512                # 4 N-tiles of 512
N_ELEM = float(D * OUT)        # elements of w
EPS = 1e-5
M_P1 = 2                       # m-tiles in the k-outer first phase
RESIDENT = 7                   # w K-tiles kept in SBUF across the scale calc


def build_kernel():
    from concourse import bacc, tile, mybir

    f32 = mybir.dt.float32
    bf16 = mybir.dt.bfloat16
    Alu = mybir.AluOpType
    Act = mybir.ActivationFunctionType
    X = mybir.AxisListType.X

    nc = bacc.Bacc(None, target_bir_lowering=False)
    x_ext = nc.declare_dram_parameter("x", [TPC, D], bf16, isOutput=False)
    w_ext = nc.declare_dram_parameter("weight", [D, OUT], f32, isOutput=False)
    out_ext = nc.declare_dram_parameter("out", [TPC, OUT], f32, isOutput=True)

    with tile.TileContext(nc) as tc:
        with (
            tc.tile_pool(name="persist", bufs=1) as persist,
            tc.tile_pool(name="wf32", bufs=10) as wf32_pool,
            tc.tile_pool(name="xbuf", bufs=4) as xbuf_pool,
            tc.tile_pool(name="sgn", bufs=4) as sgn_pool,
            tc.tile_pool(name="outp", bufs=1) as out_pool,
            tc.tile_pool(name="psum", bufs=8, space="PSUM") as psum_pool,
        ):
            wq = persist.tile([128, KT, OUT], bf16)      # quantized w^T (doubled)
            ones = persist.tile([128, 128], f32)
            partials = persist.tile([128, KT], f32)
            tot = persist.tile([128, 1], f32)
            scale_sb = persist.tile([128, 1], f32)
            t_pos = persist.tile([128, 1], f32)
            t_neg = persist.tile([128, 1], f32)
            s_half = persist.tile([128, 1], f32)

            nc.vector.memset(ones[:], 1.0)
            # PE warm-up: fetch PE's IRAM block + park the sequencer early so
            # the scale-broadcast matmul fires the moment its input is ready
            warm = psum_pool.tile([128, 512], f32, tag="psum", name="warm")
            nc.tensor.matmul(
                warm[:, 0:1], ones[:], ones[:, 0:1], start=True, stop=True
            )

            def x_dma(m):
                xb = xbuf_pool.tile([128, KT, 128], bf16, tag="xbuf", name=f"xb{m}")
                nc.sync.dma_start(
                    xb[:],
                    x_ext[m * 128 : (m + 1) * 128, :].rearrange(
                        "p (k c) -> p k c", k=KT
                    ),
                )
                return xb

            def w_dma(k, nm):
                wt = wf32_pool.tile([128, OUT], f32, tag="wf32", name=nm)
                nc.sync.dma_start(wt[:], w_ext[k * 128 : (k + 1) * 128, :])
                return wt

            # ---- stream 1: all of w, |w| row-sums; keep last RESIDENT tiles ----
            res_tiles = {}
            for k in range(KT):
                wt = w_dma(k, f"wt{k}")
                nc.vector.tensor_reduce(
                    partials[:, k : k + 1], wt[:], axis=X, op=Alu.add,
                    apply_absolute_value=True,
                )
                if k >= KT - RESIDENT:
                    res_tiles[k] = wt

            # ---- scale: sum partials, broadcast via ones-matmul ----
            nc.vector.tensor_reduce(tot[:], partials[:], axis=X, op=Alu.add)
            pbc = psum_pool.tile([128, 512], f32, tag="psum", name="pbc")
            nc.tensor.matmul(pbc[:, 0:1], ones[:], tot[:], start=True, stop=True)
            nc.vector.tensor_scalar(
                scale_sb[:], pbc[:, 0:1], 1.0 / N_ELEM, EPS, Alu.mult, Alu.max
            )
            nc.vector.tensor_scalar(t_pos[:], scale_sb[:], 1.0 / 3.0, None, Alu.mult)
            nc.vector.tensor_scalar(t_neg[:], scale_sb[:], -1.0 / 3.0, None, Alu.mult)
            nc.vector.tensor_scalar(s_half[:], scale_sb[:], 0.5, None, Alu.mult)

            # ---- quantize one K-tile (doubled ternary), hybrid ACT/DVE ----
            def quantize(k, wt):
                if k % 2 == 0 or k == 9:
                    s1 = sgn_pool.tile([128, OUT], bf16, tag="sgn", name=f"s1_{k}")
                    s2 = sgn_pool.tile([128, OUT], bf16, tag="sgn", name=f"s2_{k}")
                    nc.scalar.activation(s1[:], wt[:], Act.Sign, bias=t_pos[:, 0:1])
                    nc.scalar.activation(s2[:], wt[:], Act.Sign, bias=t_neg[:, 0:1])
                    nc.vector.tensor_tensor(wq[:, k, :], s1[:], s2[:], Alu.add)
                else:
                    neg = sgn_pool.tile([128, OUT], bf16, tag="sgn", name=f"n_{k}")
                    nc.vector.tensor_scalar(
                        wq[:, k, :], wt[:], t_pos[:, 0:1], 2.0, Alu.is_gt, Alu.mult
                    )
                    nc.vector.tensor_scalar(
                        neg[:], wt[:], t_neg[:, 0:1], 2.0, Alu.is_lt, Alu.mult
                    )
                    nc.vector.tensor_tensor(
                        wq[:, k, :], wq[:, k, :], neg[:], Alu.subtract
                    )

            # resident tiles first (data already on-chip when scale lands)
            for k in range(KT - RESIDENT, KT):
                quantize(k, res_tiles[k])

            # x for matmul phase 1, then the w re-stream interleaved with the
            # rest of x (phase-2 m order)
            xbufs = {m: x_dma(m) for m in range(M_P1)}
            xq = list(range(M_P1, M_P1 + 2))       # needed right at phase-2 start
            xlate = list(range(M_P1 + 2, MT))      # needed deep into phase-2
            for k in range(KT - RESIDENT):
                wt = w_dma(k, f"wr{k}")
                quantize(k, wt)
                if xq:
                    m_next = xq.pop(0)
                    xbufs[m_next] = x_dma(m_next)
            for m in xq + xlate:
                xbufs[m] = x_dma(m)

            # ---- matmul: out[m,n] = sum_k x[k,m].T @ wq[k,n] ----
            # phase-1 k order matches quant availability order
            korder = list(range(KT - RESIDENT, KT)) + list(range(KT - RESIDENT))

            def do_mtile(ms, ks):
                psums = [
                    psum_pool.tile([128, 512], f32, tag="psum", name=f"ps{i}")
                    for i in range(NT * len(ms))
                ]
                for ki, k in enumerate(ks):
                    for mi, m in enumerate(ms):
                        for n in range(NT):
                            nc.tensor.matmul(
                                psums[mi * NT + n][:],
                                xbufs[m][:, k, :],
                                wq[:, k, n * 512 : (n + 1) * 512],
                                start=(ki == 0),
                                stop=(ki == KT - 1),
                            )
                for mi, m in enumerate(ms):
                    ot = out_pool.tile([128, OUT], f32, tag="outp", name=f"ot{m}")
                    for n in range(NT):
                        nc.scalar.activation(
                            ot[:, n * 512 : (n + 1) * 512],
                            psums[mi * NT + n][:],
                            Act.Copy,
                            scale=s_half[:, 0:1],
                        )
                        nc.sync.dma_start(
                            out_ext[m * 128 : (m + 1) * 128, n * 512 : (n + 1) * 512],
                            ot[:, n * 512 : (n + 1) * 512],
                        )

            do_mtile(list(range(M_P1)), korder)
            for m in range(M_P1, MT):
                do_mtile([m], list(range(KT)))

    nc.finalize()
    return nc


_NC_CACHE = None


def kernel(x, weight):
    global _NC_CACHE
    import ml_dtypes
    from concourse.bass_utils import run_bass_kernel_spmd

    x = np.asarray(x, dtype=np.float32).reshape(TOK, D)
    weight = np.asarray(weight, dtype=np.float32)
    wT = np.ascontiguousarray(weight.T)                      # [in, out] f32
    in_maps = []
    for i in range(N_CORES):
        shard_t = x[i * TPC : (i + 1) * TPC].T                      # [in, tok]
        tiled = (
            shard_t.reshape(KT, 128, MT, 128)
            .transpose(2, 1, 0, 3)
            .reshape(MT * 128, KT * 128)
        )
        in_maps.append(
            {"x": np.ascontiguousarray(tiled).astype(ml_dtypes.bfloat16),
             "weight": wT}
        )

    if _NC_CACHE is None:
        _NC_CACHE = build_kernel()
    res = run_bass_kernel_spmd(_NC_CACHE, in_maps, core_ids=list(range(N_CORES)))
    outs = [res.results[i]["out"] for i in range(N_CORES)]
    return np.concatenate(outs, axis=0).reshape(B, S, OUT).astype(np.float32)


# revision 20
# speedup vs baseline: 2.8721x; 1.0025x over previous
"""BitLinear (BitNet 1.58-bit ternary) distributed Trainium2 kernel.

Reference semantics:
    scale = max(mean(|w|), 1e-5)
    w_q   = sign(w) * (|w| > scale/3)          # ternary {-1, 0, 1}
    out   = (x @ w_q.T) * scale                # x: [4, 2048, 2048], w: [2048, 2048]

Sharding: data-parallel over tokens (1024 of 8192 per core), weight
replicated; each core computes the scale locally, so there are no
collectives (cross-core sync points absorb the harness' launch skew
and invite power throttling). Host-side prep transposes both operands
so the contraction dim (in_features) lands on SBUF partitions and
pre-casts x to bf16.

scale depends on every byte of w, so w is streamed twice; the second
stream is cut to 9 of 16 K-tiles by keeping the last 7 resident in
SBUF across the scale computation (x is streamed per-m-tile through a
small rotating buffer to make room). The cross-partition total is
summed and broadcast to all 128 partitions with a single ones-matmul.
A dummy early matmul pre-fetches the PE instruction stream so the
scale-broadcast matmul fires with no dispatch latency.

Quantization: ternary, computed doubled so it is exact in bf16:
  ACT path:  wq2 = Sign(w + t) + Sign(w - t)            in {-2, 0, 2}
  DVE path:  wq2 = 2*(w > t) - 2*(w < -t)               in {-2, 0, 2}
with t = scale/3; 9 tiles on the ACT path, 7 on the DVE path to
balance engine time. The missing 1/2 is folded into the output
scaling (psum * scale/2).

Matmul: bf16 x bf16 -> fp32 PSUM, K=2048 contracted in 16 accumulating
matmuls, N=512 per PSUM bank. The first two m-tiles run k-outer across
8 PSUM banks in quant-arrival order so the PE overlaps the quant
stream; the remaining six m-tiles run as clean dense passes (~14us
each, ~97% of the warm-PE roofline).
"""

import sys

sys.path.insert(0, "/opt/trn_rl_repo")

import numpy as np

N_CORES = 8
B, S, D = 4, 2048, 2048        # x: [B, S, D]
OUT = 2048                     # out_features
TOK = B * S                    # 8192 tokens
TPC = TOK // N_CORES           # 1024 tokens per core
KT = D // 128                  # 16 K-tiles of 128
MT = TPC // 128                # 8 M-tiles per core
NT = OUT // 512                # 4 N-tiles of 512
N_ELEM = float(D * OUT)        # elements of w
EPS = 1e-5
M_P1 = 2                       # m-tiles in the k-outer first phase
RESIDENT = 7                   # w K-tiles kept in SBUF across the scale calc


def build_kernel():
    from concourse import bacc, tile, mybir

    f32 = mybir.dt.float32
    bf16 = mybir.dt.bfloat16
    Alu = mybir.AluOpType
    Act = mybir.ActivationFunctionType
    X = mybir.AxisListType.X

    nc = bacc.Bacc(None, target_bir_lowering=False)
    x_ext = nc.declare_dram_parameter("x", [TPC, D], bf16, isOutput=False)
    w_ext = nc.declare_dram_parameter("weight", [D, OUT], f32, isOutput=False)
    out_ext = nc.declare_dram_parameter("out", [TPC, OUT], f32, isOutput=True)

    with tile.TileContext(nc) as tc:
        with (
            tc.tile_pool(name="persist", bufs=1) as persist,
            tc.tile_pool(name="wf32", bufs=10) as wf32_pool,
            tc.tile_pool(name="xbuf", bufs=4) as xbuf_pool,
            tc.tile_pool(name="sgn", bufs=4) as sgn_pool,
            tc.tile_pool(name="outp", bufs=1) as out_pool,
            tc.tile_pool(name="psum", bufs=8, space="PSUM") as psum_pool,
        ):
            wq = persist.tile([128, KT, OUT], bf16)      # quantized w^T (doubled)
            ones = persist.tile([128, 128], f32)
            partials = persist.tile([128, KT], f32)
            tot = persist.tile([128, 1], f32)
            scale_sb = persist.tile([128, 1], f32)
            t_pos = persist.tile([128, 1], f32)
            t_neg = persist.tile([128, 1], f32)
            s_half = persist.tile([128, 1], f32)

            nc.vector.memset(ones[:], 1.0)
            # PE warm-up: fetch PE's IRAM block + park the sequencer early so
            # the scale-broadcast matmul fires the moment its input is ready
            warm = psum_pool.tile([128, 512], f32, tag="psum", name="warm")
            nc.tensor.matmul(
                warm[:, 0:1], ones[:], ones[:, 0:1], start=True, stop=True
            )

            def x_dma(m):
                xb = xbuf_pool.tile([128, KT, 128], bf16, tag="xbuf", name=f"xb{m}")
                nc.sync.dma_start(
                    xb[:],
                    x_ext[m * 128 : (m + 1) * 128, :].rearrange(
                        "p (k c) -> p k c", k=KT
                    ),
                )
                return xb

            def w_dma(k, nm):
                wt = wf32_pool.tile([128, OUT], f32, tag="wf32", name=nm)
                nc.sync.dma_start(wt[:], w_ext[k * 128 : (k + 1) * 128, :])
                return wt

            # ---- stream 1: all of w, |w| row-sums; keep last RESIDENT tiles ----
            res_tiles = {}
            for k in range(KT):
                wt = w_dma(k, f"wt{k}")
                nc.vector.tensor_reduce(
                    partials[:, k : k + 1], wt[:], axis=X, op=Alu.add,
                    apply_absolute_value=True,
                )
                if k >= KT - RESIDENT:
                    res_tiles[k] = wt

            # ---- scale: sum partials, broadcast via ones-matmul ----
            nc.vector.tensor_reduce(tot[:], partials[:], axis=X, op=Alu.add)
            pbc = psum_pool.tile([128, 512], f32, tag="psum", name="pbc")
            nc.tensor.matmul(pbc[:, 0:1], ones[:], tot[:], start=True, stop=True)
            nc.vector.tensor_scalar(
                scale_sb[:], pbc[:, 0:1], 1.0 / N_ELEM, EPS, Alu.mult, Alu.max
            )
            nc.vector.tensor_scalar(t_pos[:], scale_sb[:], 1.0 / 3.0, None, Alu.mult)
            nc.vector.tensor_scalar(t_neg[:], scale_sb[:], -1.0 / 3.0, None, Alu.mult)
            nc.vector.tensor_scalar(s_half[:], scale_sb[:], 0.5, None, Alu.mult)

            # ---- quantize one K-tile (doubled ternary), hybrid ACT/DVE ----
            def quantize(k, wt):
                if k % 2 == 0 or k == 9:
                    s1 = sgn_pool.tile([128, OUT], bf16, tag="sgn", name=f"s1_{k}")
                    s2 = sgn_pool.tile([128, OUT], bf16, tag="sgn", name=f"s2_{k}")
                    nc.scalar.activation(s1[:], wt[:], Act.Sign, bias=t_pos[:, 0:1])
                    nc.scalar.activation(s2[:], wt[:], Act.Sign, bias=t_neg[:, 0:1])
                    nc.vector.tensor_tensor(wq[:, k, :], s1[:], s2[:], Alu.add)
                else:
                    neg = sgn_pool.tile([128, OUT], bf16, tag="sgn", name=f"n_{k}")
                    nc.vector.tensor_scalar(
                        wq[:, k, :], wt[:], t_pos[:, 0:1], 2.0, Alu.is_gt, Alu.mult
                    )
                    nc.vector.tensor_scalar(
                        neg[:], wt[:], t_neg[:, 0:1], 2.0, Alu.is_lt, Alu.mult
                    )
                    nc.vector.tensor_tensor(
                        wq[:, k, :], wq[:, k, :], neg[:], Alu.subtract
                    )

            # resident tiles first (data already on-chip when scale lands)
            for k in range(KT - RESIDENT, KT):
                quantize(k, res_tiles[k])

            # x for matmul phase 1, then the w re-stream interleaved with the
            # rest of x (phase-2 m order)
            xbufs = {m: x_dma(m) for m in range(M_P1)}
            xq = list(range(M_P1, M_P1 + 2))       # needed right at phase-2 start
            xlate = list(range(M_P1 + 2, MT))      # needed deep into phase-2
            for k in range(KT - RESIDENT):
                wt = w_dma(k, f"wr{k}")
                quantize(k, wt)
                if xq:
                    m_next = xq.pop(0)
                    xbufs[m_next] = x_dma(m_next)
            for m in xq + xlate:
                xbufs[m] = x_dma(m)

            # ---- matmul: out[m,n] = sum_k x[k,m].T @ wq[k,n] ----
            # phase-1 k order matches quant availability order
            korder = list(range(KT - RESIDENT, KT)) + list(range(KT - RESIDENT))

            def do_mtile(ms, ks):
                psums = [
                    psum_pool.tile([128, 512], f32, tag="psum", name=f"ps{i}")
                    for i in range(NT * len(ms))
                ]
                for ki, k in enumerate(ks):
                    for mi, m in enumerate(ms):
                        for n in range(NT):
                            nc.tensor.matmul(
                                psums[mi * NT + n][:],
                                xbufs[m][:, k, :],
                                wq[:, k, n * 512 : (n + 1) * 512],
                                start=(ki == 0),
                                stop=(ki == KT - 1),
                            )
                for mi, m in enumerate(ms):
                    ot = out_pool.tile([128, OUT], f32, tag="outp", name=f"ot{m}")
                    for n in range(NT):
                        nc.scalar.activation(
                            ot[:, n * 512 : (n + 1) * 512],
                            psums[mi * NT + n][:],
                            Act.Copy,
                            scale=s_half[:, 0:1],
                        )
                        nc.sync.dma_start(
                            out_ext[m * 128 : (m + 1) * 128, n * 512 : (n + 1) * 512],
                            ot[:, n * 512 : (n + 1) * 512],
                        )

            do_mtile(list(range(M_P1)), korder)
            for m in range(M_P1, MT):
                do_mtile([m], list(range(KT)))

    nc.finalize()
    return nc


_NC_CACHE = None


def kernel(x, weight):
    global _NC_CACHE
    import ml_dtypes
    from concourse.bass_utils import run_bass_kernel_spmd

    x = np.asarray(x, dtype=np.float32).reshape(TOK, D)
    weight = np.asarray(weight, dtype=np.float32)
    wT = np.ascontiguousarray(weight.T)                      # [in, out] f32
    in_maps = []
    for i in range(N_CORES):
        shard_t = x[i * TPC : (i + 1) * TPC].T                      # [in, tok]
        tiled = (
            shard_t.reshape(KT, 128, MT, 128)
            .transpose(2, 1, 0, 3)
            .reshape(MT * 128, KT * 128)
        )
        in_maps.append(
            {"x": np.ascontiguousarray(tiled).astype(ml_dtypes.bfloat16),
             "weight": wT}
        )

    if _NC_CACHE is None:
        _NC_CACHE = build_kernel()
    res = run_bass_kernel_spmd(_NC_CACHE, in_maps, core_ids=list(range(N_CORES)))
    outs = [res.results[i]["out"] for i in range(N_CORES)]
    return np.concatenate(outs, axis=0).reshape(B, S, OUT).astype(np.float32)
